# revision 8
# baseline (speedup 1.0000x reference)
"""Trainium2 Bass kernel for CustomAttention (B=4, N=2048, C=1024, H=16).

Sharding: 8-way tensor-parallel over heads (2 heads per core, all batches).
Each core computes qkv projection for its head slice, full attention for its
(batch, head) pairs, and a partial output projection over its 128 channels.
Host sums the 8 partial projections and adds proj_b.

v2 design (vs v1): bf16 matmuls and bf16 HBM I/O; host precomputes
exp(attn_bias) so softmax is p = exp(s) * eb (bf16 SBUF mul at DVE 2x mode)
instead of an fp32 PSUM bias-add at 1x; qkv biases folded into K=1 ones-row
matmuls; v produced directly in [token, dim] layout (no PE transposes);
per-batch software pipeline so qkv(b+1)/proj(b-1) overlap attention(b).

Per-core layouts (host-prepped):
  xT    [1024, 8192] bf16  x reshaped [B*N, C], transposed (same all cores)
  wqkv  [1024, 384]  bf16  qkv weight rows for (q,k,v) x (2 heads x 64),
                           transposed; q rows pre-scaled by 1/sqrt(D)
  bqkv  [1, 384]     bf16  matching bias (q part pre-scaled)
  ebT   [2, 2048, 2048] bf16  exp(attn_bias[h])[n, m] transposed to [m, n]
  pw    [128, 1024]  bf16  proj_w columns for this core's 128 channels, T
Output:
  outT  [1024, 8192] bf16  partial (attn_out @ proj_w_slice) transposed;
                           host sums partials in fp32, adds proj_b.
"""

import sys

if "/opt/trn_rl_repo" not in sys.path:
    sys.path.insert(0, "/opt/trn_rl_repo")

import numpy as np

B, N, C, H, D = 4, 2048, 1024, 16, 64
T = B * N  # 8192
HPC = 2  # heads per core
NCORES = 8
MB = N // 128  # 16 key blocks per batch
MBP = MB // 2  # 8 key-block pairs
NCH = N // 512  # 4 query chunks of 512 per batch
TCH = N // 512  # 4 token chunks per batch (qkv phase)
KC = C // 128  # 8 contraction chunks (qkv phase)
JC = C // 128  # 8 output-channel chunks (proj phase)
DA = D + 1  # 65: head dim + ones row for denominator

_CACHE = {}


def build_nc():
    import concourse.bass as bass
    import concourse.bacc as bacc
    import concourse.mybir as mybir
    import concourse.tile as tile
    from contextlib import ExitStack

    F32 = mybir.dt.float32
    BF16 = mybir.dt.bfloat16
    EXP = mybir.ActivationFunctionType.Exp

    nc = bacc.Bacc(None, target_bir_lowering=False)
    xT = nc.dram_tensor("xT", [C, T], BF16, kind="ExternalInput")
    wqkv = nc.dram_tensor("wqkv", [C, 3 * HPC * D], BF16, kind="ExternalInput")
    bqkv = nc.dram_tensor("bqkv", [1, 3 * HPC * D], BF16, kind="ExternalInput")
    ebT = nc.dram_tensor("ebT", [HPC, N, N], BF16, kind="ExternalInput")
    pw = nc.dram_tensor("pw", [HPC * D, C], BF16, kind="ExternalInput")
    outT = nc.dram_tensor("outT", [C, T], BF16, kind="ExternalOutput")

    with tile.TileContext(nc) as tc, ExitStack() as ctx:
        sing = ctx.enter_context(tc.tile_pool(name="sing", bufs=1))
        ps = ctx.enter_context(tc.tile_pool(name="ps", bufs=1, space="PSUM"))
        work = ctx.enter_context(tc.tile_pool(name="work", bufs=1))

        # ---- residents ----
        w_sb = sing.tile([128, KC, 3 * HPC * D], BF16)
        nc.sync.dma_start(out=w_sb, in_=wqkv.rearrange("(k p) m -> p k m", p=128))
        bq_sb = sing.tile([1, 3 * HPC * D], BF16)
        nc.sync.dma_start(out=bq_sb, in_=bqkv[:, :])
        pw_sb = sing.tile([128, C], BF16)
        nc.sync.dma_start(out=pw_sb, in_=pw[:, :])
        ones_row = sing.tile([1, 512], BF16)
        nc.vector.memset(ones_row, 1.0)

        # exp(bias), resident: [p, mb, h, n] (loaded after qkv(0) DMAs)
        eb_sb = sing.tile([128, MB, HPC, N], BF16)

        def load_eb():
            for mb in range(MB):
                for h in range(HPC):
                    nc.sync.dma_start(
                        out=eb_sb[:, mb, h],
                        in_=ebT[h, mb * 128 : (mb + 1) * 128, :],
                    )

        # double-buffered (b%2) per-batch residents
        qT = sing.tile([128, 2, N], BF16)  # rows: q_h0 d0..63 | q_h1 d0..63
        kT = sing.tile([128, 2, N], BF16)
        # v_aug[:, s, mb, :]: cols 0:64 v_h0, col 64 ones, 65:129 v_h1, 129 ones
        v_aug = sing.tile([128, 2, MB, 2 * DA], BF16)
        ones_cols = sing.tile([128, 2, MB], BF16)
        nc.vector.memset(ones_cols, 1.0)
        nc.vector.tensor_copy(v_aug[:, :, :, D], ones_cols)
        nc.vector.tensor_copy(v_aug[:, :, :, 2 * D + 1], ones_cols)
        attn_o = sing.tile([128, N], BF16)

        def qkv_phase(b):
            """qkv projection for batch b into qT/kT/v_aug slot b%2."""
            s = b % 2
            bo = b * N
            for t in range(TCH):
                t0 = bo + t * 512
                x_tiles = []
                for kc in range(KC):
                    x_t = work.tile([128, 512], BF16, tag="x", bufs=KC)
                    nc.sync.dma_start(out=x_t, in_=xT[kc * 128 : (kc + 1) * 128, t0 : t0 + 512])
                    x_tiles.append(x_t)
                # q and k: out [dim 128, tok 512]
                for m in range(2):
                    mm_ps = ps.tile([128, 512], F32, tag="mm", bufs=2)
                    for kc in range(KC):
                        nc.tensor.matmul(
                            mm_ps,
                            w_sb[:, kc, m * 128 : (m + 1) * 128],
                            x_tiles[kc],
                            start=(kc == 0),
                            stop=False,
                        )
                    # bias via ones-row: out[dim, tok] += bq[dim] * 1
                    nc.tensor.matmul(
                        mm_ps,
                        bq_sb[0:1, m * 128 : (m + 1) * 128],
                        ones_row[0:1, :],
                        start=False,
                        stop=True,
                    )
                    dst = qT if m == 0 else kT
                    nc.vector.tensor_copy(dst[:, s, t * 512 : (t + 1) * 512], mm_ps)
                # v: out [tok 128, dim 128] per 128-token block
                for j in range(4):
                    mb = t * 4 + j
                    v_ps = ps.tile([128, 512], F32, tag="mm", bufs=2)
                    for kc in range(KC):
                        nc.tensor.matmul(
                            v_ps[:, 0:128],
                            x_tiles[kc][:, j * 128 : (j + 1) * 128],
                            w_sb[:, kc, 256:384],
                            start=(kc == 0),
                            stop=False,
                        )
                    nc.tensor.matmul(
                        v_ps[:, 0:128],
                        ones_row[0:1, 0:128],
                        bq_sb[0:1, 256:384],
                        start=False,
                        stop=True,
                    )
                    nc.vector.tensor_copy(v_aug[:, s, mb, 0:D], v_ps[:, 0:D])
                    nc.vector.tensor_copy(
                        v_aug[:, s, mb, DA : DA + D], v_ps[:, D : 2 * D]
                    )

        def attn_phase(b):
            s = b % 2
            for ncq in range(NCH):
                n0 = ncq * 512
                pv = [
                    ps.tile([DA, 512], F32, tag=f"pv{h}", bufs=1, name=f"pv_{b}_{ncq}_{h}")
                    for h in range(HPC)
                ]
                for mbp in range(MBP):
                    # scores for key-block pair (2*mbp, 2*mbp+1), both heads
                    s_ps = [
                        ps.tile([128, 2, 512], F32, tag=f"s{h}", bufs=1, name=f"s_{b}_{ncq}_{mbp}_{h}")
                        for h in range(HPC)
                    ]
                    for mbi in range(2):
                        m0 = (2 * mbp + mbi) * 128
                        for h in range(HPC):
                            hd = h * D
                            nc.tensor.matmul(
                                s_ps[h][:, mbi, :],
                                kT[hd : hd + D, s, m0 : m0 + 128],
                                qT[hd : hd + D, s, n0 : n0 + 512],
                                start=True,
                                stop=True,
                            )
                    e_sb = work.tile([128, 2, HPC, 512], BF16, tag="exp", bufs=3)
                    for h in range(HPC):
                        nc.scalar.activation(e_sb[:, :, h, :], s_ps[h], EXP)
                    nc.vector.tensor_mul(
                        e_sb,
                        e_sb,
                        eb_sb[:, 2 * mbp : 2 * mbp + 2, :, n0 : n0 + 512],
                    )
                    for h in range(HPC):
                        for mbi in range(2):
                            mb = 2 * mbp + mbi
                            nc.tensor.matmul(
                                pv[h],
                                v_aug[:, s, mb, h * DA : (h + 1) * DA],
                                e_sb[:, mbi, h, :],
                                start=(mbp == 0 and mbi == 0),
                                stop=(mbp == MBP - 1 and mbi == 1),
                            )
                # normalize: attn_o[h rows, n0:n0+512] = pv[0:64] / pv[64]
                den = work.tile([DA, 2, 512], F32, tag="den", bufs=1)
                for h in range(HPC):
                    nc.vector.reciprocal(den[D : D + 1, h], pv[h][D : D + 1, :])
                rec0 = work.tile([1, 2, 512], F32, tag="rec0", bufs=1)
                nc.sync.dma_start(out=rec0, in_=den[D : D + 1])
                rbc = work.tile([D, 2, 512], F32, tag="rbc", bufs=1)
                nc.gpsimd.partition_broadcast(rbc, rec0)
                nc.vector.tensor_mul(
                    attn_o[0:D, n0 : n0 + 512], pv[0][0:D, :], rbc[:, 0]
                )
                tmp = work.tile([D, 512], BF16, tag="tmp", bufs=2)
                nc.vector.tensor_mul(tmp, pv[1][0:D, :], rbc[:, 1])
                nc.sync.dma_start(out=attn_o[D : 2 * D, n0 : n0 + 512], in_=tmp)

        def proj_phase(b):
            s = b % 2
            bo = b * N
            for jc in range(JC):
                for ncq in range(NCH):
                    n0 = ncq * 512
                    pr_ps = ps.tile([128, 512], F32, tag="mm", bufs=2)
                    nc.tensor.matmul(
                        pr_ps,
                        pw_sb[:, jc * 128 : (jc + 1) * 128],
                        attn_o[:, n0 : n0 + 512],
                        start=True,
                        stop=True,
                    )
                    o_sb = work.tile([128, 512], BF16, tag="o", bufs=2)
                    nc.vector.tensor_copy(o_sb, pr_ps)
                    nc.sync.dma_start(
                        out=outT[jc * 128 : (jc + 1) * 128, bo + n0 : bo + n0 + 512],
                        in_=o_sb,
                    )

        # software pipeline: qkv(b+1) and proj(b-1) overlap attention(b)
        qkv_phase(0)
        load_eb()
        for b in range(B):
            attn_phase(b)
            if b + 1 < B:
                qkv_phase(b + 1)
            proj_phase(b)

    nc.compile()
    return nc


def _get_nc():
    if "nc" not in _CACHE:
        _CACHE["nc"] = build_nc()
    return _CACHE["nc"]


def make_in_maps(x, attn_bias, qkv_w, qkv_b, proj_w):
    import ml_dtypes

    BF = ml_dtypes.bfloat16
    x = np.asarray(x, dtype=np.float32)
    attn_bias = np.asarray(attn_bias, dtype=np.float32)
    qkv_w = np.asarray(qkv_w, dtype=np.float32)
    qkv_b = np.asarray(qkv_b, dtype=np.float32)
    proj_w = np.asarray(proj_w, dtype=np.float32)

    xT = np.ascontiguousarray(x.reshape(T, C).T.astype(BF))
    # exp(bias) per head, transposed to [m, n]
    ebT_full = np.exp(attn_bias[0]).transpose(0, 2, 1).astype(BF)
    scale = 1.0 / np.sqrt(D)

    in_maps = []
    for cid in range(NCORES):
        h0 = HPC * cid
        rows = np.r_[h0 * D : (h0 + 2) * D]
        wq = qkv_w[rows, :] * scale
        wk = qkv_w[C + rows, :]
        wv = qkv_w[2 * C + rows, :]
        wqkv_c = np.ascontiguousarray(
            np.concatenate([wq, wk, wv], 0).T.astype(BF)
        )
        bq = qkv_b[rows] * scale
        bk = qkv_b[C + rows]
        bv = qkv_b[2 * C + rows]
        bqkv_c = np.ascontiguousarray(
            np.concatenate([bq, bk, bv], 0).astype(BF)[None, :]
        )
        ebT_c = np.ascontiguousarray(ebT_full[h0 : h0 + HPC])
        pw_c = np.ascontiguousarray(
            proj_w[:, cid * 128 : (cid + 1) * 128].T.astype(BF)
        )
        in_maps.append(
            {"xT": xT, "wqkv": wqkv_c, "bqkv": bqkv_c, "ebT": ebT_c, "pw": pw_c}
        )
    return in_maps


def combine_outputs(partials, proj_b):
    proj_b = np.asarray(proj_b, dtype=np.float32)
    acc = partials[0].astype(np.float32)
    for p in partials[1:]:
        acc += p.astype(np.float32)
    out = acc.T + proj_b[None, :]
    return np.ascontiguousarray(out.reshape(B, N, C).astype(np.float32))


def kernel(x, attn_bias, qkv_w, qkv_b, proj_w, proj_b):
    from concourse.bass_utils import run_bass_kernel_spmd

    in_maps = make_in_maps(x, attn_bias, qkv_w, qkv_b, proj_w)
    res = run_bass_kernel_spmd(_get_nc(), in_maps, core_ids=list(range(NCORES)))
    partials = [res.results[i]["outT"] for i in range(NCORES)]
    return combine_outputs(partials, proj_b)


# revision 9
# speedup vs baseline: 14.1763x; 14.1763x over previous
"""Trainium2 Bass kernel for CustomAttention (B=4, N=2048, C=1024, H=16).

Sharding: 8-way tensor-parallel over heads (2 heads per core, all batches).
Each core computes qkv projection for its head slice, full attention for its
(batch, head) pairs, and a partial output projection over its 128 channels.
Host sums the 8 partial projections and adds proj_b.

v2 design (vs v1): bf16 matmuls and bf16 HBM I/O; host precomputes
exp(attn_bias) so softmax is p = exp(s) * eb (bf16 SBUF mul at DVE 2x mode)
instead of an fp32 PSUM bias-add at 1x; qkv biases folded into K=1 ones-row
matmuls; v produced directly in [token, dim] layout (no PE transposes);
per-batch software pipeline so qkv(b+1)/proj(b-1) overlap attention(b).

Per-core layouts (host-prepped):
  xT    [1024, 8192] bf16  x reshaped [B*N, C], transposed (same all cores)
  wqkv  [1024, 384]  bf16  qkv weight rows for (q,k,v) x (2 heads x 64),
                           transposed; q rows pre-scaled by 1/sqrt(D)
  bqkv  [1, 384]     bf16  matching bias (q part pre-scaled)
  ebT   [2, 2048, 2048] bf16  exp(attn_bias[h])[n, m] transposed to [m, n]
  pw    [128, 1024]  bf16  proj_w columns for this core's 128 channels, T
Output:
  outT  [1024, 8192] bf16  partial (attn_out @ proj_w_slice) transposed;
                           host sums partials in fp32, adds proj_b.
"""

import sys

if "/opt/trn_rl_repo" not in sys.path:
    sys.path.insert(0, "/opt/trn_rl_repo")

import numpy as np

B, N, C, H, D = 4, 2048, 1024, 16, 64
T = B * N  # 8192
HPC = 2  # heads per core
NCORES = 8
MB = N // 128  # 16 key blocks per batch
MBP = MB // 2  # 8 key-block pairs
NCH = N // 512  # 4 query chunks of 512 per batch
TCH = N // 512  # 4 token chunks per batch (qkv phase)
KC = C // 128  # 8 contraction chunks (qkv phase)
JC = C // 128  # 8 output-channel chunks (proj phase)
DA = D + 1  # 65: head dim + ones row for denominator

_CACHE = {}


def build_nc():
    import concourse.bass as bass
    import concourse.bacc as bacc
    import concourse.mybir as mybir
    import concourse.tile as tile
    from contextlib import ExitStack

    F32 = mybir.dt.float32
    BF16 = mybir.dt.bfloat16
    EXP = mybir.ActivationFunctionType.Exp

    nc = bacc.Bacc(None, target_bir_lowering=False)
    xT = nc.dram_tensor("xT", [C, T], BF16, kind="ExternalInput")
    wqkv = nc.dram_tensor("wqkv", [C, 3 * HPC * D], BF16, kind="ExternalInput")
    bqkv = nc.dram_tensor("bqkv", [1, 3 * HPC * D], BF16, kind="ExternalInput")
    ebT = nc.dram_tensor("ebT", [HPC, N, N], BF16, kind="ExternalInput")
    pw = nc.dram_tensor("pw", [HPC * D, C], BF16, kind="ExternalInput")
    outT = nc.dram_tensor("outT", [C, T], BF16, kind="ExternalOutput")

    with tile.TileContext(nc) as tc, ExitStack() as ctx:
        sing = ctx.enter_context(tc.tile_pool(name="sing", bufs=1))
        ps = ctx.enter_context(tc.tile_pool(name="ps", bufs=1, space="PSUM"))
        work = ctx.enter_context(tc.tile_pool(name="work", bufs=1))

        # ---- residents ----
        w_sb = sing.tile([128, KC, 3 * HPC * D], BF16)
        nc.sync.dma_start(out=w_sb, in_=wqkv.rearrange("(k p) m -> p k m", p=128))
        bq_sb = sing.tile([1, 3 * HPC * D], BF16)
        nc.sync.dma_start(out=bq_sb, in_=bqkv[:, :])
        pw_sb = sing.tile([128, C], BF16)
        nc.sync.dma_start(out=pw_sb, in_=pw[:, :])
        ones_row = sing.tile([1, 512], BF16)
        nc.vector.memset(ones_row, 1.0)

        # exp(bias), resident: [p, mb, h, n] (loaded after qkv(0) DMAs)
        eb_sb = sing.tile([128, MB, HPC, N], BF16)

        def load_eb():
            for mb in range(MB):
                nc.sync.dma_start(
                    out=eb_sb[:, mb],
                    in_=ebT[:, mb * 128 : (mb + 1) * 128, :].rearrange(
                        "h p n -> p h n"
                    ),
                )

        # double-buffered (b%2) per-batch residents
        qT = sing.tile([128, 2, N], BF16)  # rows: q_h0 d0..63 | q_h1 d0..63
        kT = sing.tile([128, 2, N], BF16)
        # v_aug[:, s, mb, :]: cols 0:64 v_h0, col 64 ones, 65:129 v_h1, 129 ones
        v_aug = sing.tile([128, 2, MB, 2 * DA], BF16)
        ones_cols = sing.tile([128, 2, MB], BF16)
        nc.vector.memset(ones_cols, 1.0)
        nc.vector.tensor_copy(v_aug[:, :, :, D], ones_cols)
        nc.vector.tensor_copy(v_aug[:, :, :, 2 * D + 1], ones_cols)
        attn_o = sing.tile([128, N], BF16)

        def qkv_phase(b):
            """qkv projection for batch b into qT/kT/v_aug slot b%2."""
            s = b % 2
            bo = b * N
            for t in range(TCH):
                t0 = bo + t * 512
                x_tiles = []
                for kc in range(KC):
                    x_t = work.tile([128, 512], BF16, tag="x", bufs=KC)
                    nc.sync.dma_start(out=x_t, in_=xT[kc * 128 : (kc + 1) * 128, t0 : t0 + 512])
                    x_tiles.append(x_t)
                # q and k: out [dim 128, tok 512]
                for m in range(2):
                    mm_ps = ps.tile([128, 512], F32, tag="mm", bufs=2)
                    for kc in range(KC):
                        nc.tensor.matmul(
                            mm_ps,
                            w_sb[:, kc, m * 128 : (m + 1) * 128],
                            x_tiles[kc],
                            start=(kc == 0),
                            stop=False,
                        )
                    # bias via ones-row: out[dim, tok] += bq[dim] * 1
                    nc.tensor.matmul(
                        mm_ps,
                        bq_sb[0:1, m * 128 : (m + 1) * 128],
                        ones_row[0:1, :],
                        start=False,
                        stop=True,
                    )
                    dst = qT if m == 0 else kT
                    nc.vector.tensor_copy(dst[:, s, t * 512 : (t + 1) * 512], mm_ps)
                # v: out [tok 128, dim 128] per 128-token block
                for j in range(4):
                    mb = t * 4 + j
                    v_ps = ps.tile([128, 512], F32, tag="mm", bufs=2)
                    for kc in range(KC):
                        nc.tensor.matmul(
                            v_ps[:, 0:128],
                            x_tiles[kc][:, j * 128 : (j + 1) * 128],
                            w_sb[:, kc, 256:384],
                            start=(kc == 0),
                            stop=False,
                        )
                    nc.tensor.matmul(
                        v_ps[:, 0:128],
                        ones_row[0:1, 0:128],
                        bq_sb[0:1, 256:384],
                        start=False,
                        stop=True,
                    )
                    nc.vector.tensor_copy(v_aug[:, s, mb, 0:D], v_ps[:, 0:D])
                    nc.vector.tensor_copy(
                        v_aug[:, s, mb, DA : DA + D], v_ps[:, D : 2 * D]
                    )

        def attn_phase(b):
            s = b % 2
            for ncq in range(NCH):
                n0 = ncq * 512
                pv = [
                    ps.tile([DA, 512], F32, tag=f"pv{h}", bufs=1, name=f"pv_{b}_{ncq}_{h}")
                    for h in range(HPC)
                ]
                for mbp in range(MBP):
                    # scores for key-block pair (2*mbp, 2*mbp+1), both heads
                    s_ps = [
                        ps.tile([128, 2, 512], F32, tag=f"s{h}", bufs=1, name=f"s_{b}_{ncq}_{mbp}_{h}")
                        for h in range(HPC)
                    ]
                    for mbi in range(2):
                        m0 = (2 * mbp + mbi) * 128
                        for h in range(HPC):
                            hd = h * D
                            nc.tensor.matmul(
                                s_ps[h][:, mbi, :],
                                kT[hd : hd + D, s, m0 : m0 + 128],
                                qT[hd : hd + D, s, n0 : n0 + 512],
                                start=True,
                                stop=True,
                            )
                    e_sb = work.tile([128, 2, HPC, 512], BF16, tag="exp", bufs=3)
                    for h in range(HPC):
                        nc.scalar.activation(e_sb[:, :, h, :], s_ps[h], EXP)
                    nc.vector.tensor_mul(
                        e_sb,
                        e_sb,
                        eb_sb[:, 2 * mbp : 2 * mbp + 2, :, n0 : n0 + 512],
                    )
                    for h in range(HPC):
                        for mbi in range(2):
                            mb = 2 * mbp + mbi
                            nc.tensor.matmul(
                                pv[h],
                                v_aug[:, s, mb, h * DA : (h + 1) * DA],
                                e_sb[:, mbi, h, :],
                                start=(mbp == 0 and mbi == 0),
                                stop=(mbp == MBP - 1 and mbi == 1),
                            )
                # normalize: attn_o[h rows, n0:n0+512] = pv[0:64] / pv[64]
                den = work.tile([DA, 2, 512], F32, tag="den", bufs=1)
                for h in range(HPC):
                    nc.vector.reciprocal(den[D : D + 1, h], pv[h][D : D + 1, :])
                rec0 = work.tile([1, 2, 512], F32, tag="rec0", bufs=1)
                nc.sync.dma_start(out=rec0, in_=den[D : D + 1])
                rbc = work.tile([D, 2, 512], F32, tag="rbc", bufs=1)
                nc.gpsimd.partition_broadcast(rbc, rec0)
                nc.vector.tensor_mul(
                    attn_o[0:D, n0 : n0 + 512], pv[0][0:D, :], rbc[:, 0]
                )
                tmp = work.tile([D, 512], BF16, tag="tmp", bufs=2)
                nc.vector.tensor_mul(tmp, pv[1][0:D, :], rbc[:, 1])
                nc.sync.dma_start(out=attn_o[D : 2 * D, n0 : n0 + 512], in_=tmp)

        def proj_phase(b):
            s = b % 2
            bo = b * N
            for jc in range(JC):
                for ncq in range(NCH):
                    n0 = ncq * 512
                    pr_ps = ps.tile([128, 512], F32, tag="mm", bufs=2)
                    nc.tensor.matmul(
                        pr_ps,
                        pw_sb[:, jc * 128 : (jc + 1) * 128],
                        attn_o[:, n0 : n0 + 512],
                        start=True,
                        stop=True,
                    )
                    o_sb = work.tile([128, 512], BF16, tag="o", bufs=2)
                    nc.vector.tensor_copy(o_sb, pr_ps)
                    nc.sync.dma_start(
                        out=outT[jc * 128 : (jc + 1) * 128, bo + n0 : bo + n0 + 512],
                        in_=o_sb,
                    )

        # software pipeline: qkv(b+1) and proj(b-1) overlap attention(b)
        qkv_phase(0)
        load_eb()
        for b in range(B):
            attn_phase(b)
            if b + 1 < B:
                qkv_phase(b + 1)
            proj_phase(b)

    nc.compile()
    return nc


def _get_nc():
    if "nc" not in _CACHE:
        _CACHE["nc"] = build_nc()
    return _CACHE["nc"]


def make_in_maps(x, attn_bias, qkv_w, qkv_b, proj_w):
    import ml_dtypes

    BF = ml_dtypes.bfloat16
    x = np.asarray(x, dtype=np.float32)
    attn_bias = np.asarray(attn_bias, dtype=np.float32)
    qkv_w = np.asarray(qkv_w, dtype=np.float32)
    qkv_b = np.asarray(qkv_b, dtype=np.float32)
    proj_w = np.asarray(proj_w, dtype=np.float32)

    xT = np.ascontiguousarray(x.reshape(T, C).T.astype(BF))
    # exp(bias) per head, transposed to [m, n]
    ebT_full = np.exp(attn_bias[0]).transpose(0, 2, 1).astype(BF)
    scale = 1.0 / np.sqrt(D)

    in_maps = []
    for cid in range(NCORES):
        h0 = HPC * cid
        rows = np.r_[h0 * D : (h0 + 2) * D]
        wq = qkv_w[rows, :] * scale
        wk = qkv_w[C + rows, :]
        wv = qkv_w[2 * C + rows, :]
        wqkv_c = np.ascontiguousarray(
            np.concatenate([wq, wk, wv], 0).T.astype(BF)
        )
        bq = qkv_b[rows] * scale
        bk = qkv_b[C + rows]
        bv = qkv_b[2 * C + rows]
        bqkv_c = np.ascontiguousarray(
            np.concatenate([bq, bk, bv], 0).astype(BF)[None, :]
        )
        ebT_c = np.ascontiguousarray(ebT_full[h0 : h0 + HPC])
        pw_c = np.ascontiguousarray(
            proj_w[:, cid * 128 : (cid + 1) * 128].T.astype(BF)
        )
        in_maps.append(
            {"xT": xT, "wqkv": wqkv_c, "bqkv": bqkv_c, "ebT": ebT_c, "pw": pw_c}
        )
    return in_maps


def combine_outputs(partials, proj_b):
    proj_b = np.asarray(proj_b, dtype=np.float32)
    acc = partials[0].astype(np.float32)
    for p in partials[1:]:
        acc += p.astype(np.float32)
    out = acc.T + proj_b[None, :]
    return np.ascontiguousarray(out.reshape(B, N, C).astype(np.float32))


def kernel(x, attn_bias, qkv_w, qkv_b, proj_w, proj_b):
    from concourse.bass_utils import run_bass_kernel_spmd

    in_maps = make_in_maps(x, attn_bias, qkv_w, qkv_b, proj_w)
    res = run_bass_kernel_spmd(_get_nc(), in_maps, core_ids=list(range(NCORES)))
    partials = [res.results[i]["outT"] for i in range(NCORES)]
    return combine_outputs(partials, proj_b)
